# revision 28
# baseline (speedup 1.0000x reference)
"""Multi-head attention block (QKV proj + softmax attention + out proj) on 8
Trainium2 NeuronCores.

Problem shapes: x [4, 1024, 1024], Wqkv [3072, 1024], bqkv [3072],
W1 [1024, 1024], b1 [1024].  out = Attention(x) @ W1.T + b1, 16 heads, d=64,
softmax scale 1/sqrt(1024) = 1/32.

Sharding: core c handles batch b = c // 2 and head-group hg = c % 2 (8 of the
16 heads).  Each core computes its heads' QKV projection, full attention for
those heads over its batch, and a *partial* output projection against the
W1 columns its heads feed.  The host sums the two partials per batch and adds
b1.  No device collectives.

Structure (v7):
  - S psum fused per (pair, jc) into one [128, 2048] tile (4 banks) covering
    both heads x both query halves; ONE exp per group amortizes the ACT
    PSUM-read bubble and halves the number of chain round-trips; the 4 S
    matmuls issue back-to-back so the row-tiled (tile_position 0/64) pairs
    run concurrently on the PE.
  - S+exp emitted under tc.high_priority() so the static scheduler never
    parks filler matmuls ahead of the pacing chain (PE FIFO is strict).
  - Window balancing: qk_proj(2)/qk_proj(3)/pv chains are dep-gated to the
    attention window where their PE time is needed; otherwise the scheduler
    drains all projection work early and the PE starves (and HAM-cools) in
    the later windows.
  - Host reorders Wqkv columns pair-major ([q0 k0 q1 k1 ... v]) so the
    pair-0 slice is one contiguous early DMA; input DMAs are split into
    gate-critical (x, qk pair-0) vs deferred (rest) sets balanced over the
    3 DMA queues, which pulls the first exp ~13us earlier.
  - Tail: all 16 output-projection groups on psP, psum->bf16 casts split
    DVE/ACT, output DMAs rotated over the 3 queues.
"""

import numpy as np

B = 4
N = 1024            # tokens per batch
DIM = 1024          # model dim
HEADS = 16
D = DIM // HEADS    # 64
NCORES = 8
HG = 2              # head groups (tensor-parallel degree over heads)
NHL = HEADS // HG   # 8 local heads
FQ = NHL * D        # 512 local q (or k or v) features
FT = 3 * FQ         # 1536 local qkv features
P = 128
TH = 512            # token half (matmul free dim)

_CACHE = {}
MM_DTYPE = "bfloat16"


def _build(mm_dtype=None):
    if mm_dtype is None:
        mm_dtype = MM_DTYPE
    from contextlib import ExitStack

    import concourse.bacc as bacc
    import concourse.bass as bass
    import concourse.tile as tile
    from concourse import mybir
    from concourse.tile import add_dep_helper

    f32 = mybir.dt.float32
    mmdt = getattr(mybir.dt, mm_dtype)

    nc = bacc.Bacc("TRN2", target_bir_lowering=False)

    xT = nc.dram_tensor("xT", [DIM, N], mmdt, kind="ExternalInput")
    wqkvT = nc.dram_tensor("wqkvT", [DIM, FT], mmdt, kind="ExternalInput")
    bqkvT = nc.dram_tensor("bqkvT", [P, FT // P], f32, kind="ExternalInput")
    bv = nc.dram_tensor("bv", [FQ], f32, kind="ExternalInput")
    w1T = nc.dram_tensor("w1T", [FQ, DIM], mmdt, kind="ExternalInput")
    outdt = mmdt if mm_dtype == "bfloat16" else f32
    outT = nc.dram_tensor("outT", [DIM, N], outdt, kind="ExternalOutput")

    Exp = mybir.ActivationFunctionType.Exp

    with tile.TileContext(nc) as tc, ExitStack() as ctx:
        const = ctx.enter_context(tc.tile_pool(name="const", bufs=1))
        psS = ctx.enter_context(tc.tile_pool(name="psS", bufs=1, space="PSUM"))
        psP = ctx.enter_context(tc.tile_pool(name="psP", bufs=2, space="PSUM"))
        psB = ctx.enter_context(tc.tile_pool(name="psB", bufs=2, space="PSUM"))
        outp = ctx.enter_context(tc.tile_pool(name="outp", bufs=6))
        small = ctx.enter_context(tc.tile_pool(name="small", bufs=4))
        loadp = ctx.enter_context(tc.tile_pool(name="loadp", bufs=1))

        # persistent SBUF
        qt = const.tile([P, 4, N], mmdt)        # Q.T  [f-inner, head-pair, tok]
        kt = const.tile([P, 4, N], mmdt)        # K.T
        vs = const.tile([P, 8, NHL * 65], mmdt)  # V'  [tok-inner, j-chunk, h*65+e]
        at = const.tile([P, 4, N], mmdt)        # A.T  [f-inner, f-chunk, tok]
        w1s = const.tile([P, 4, DIM], mmdt)     # W1loc.T [f-inner, f-chunk, out]
        bqv = const.tile([P, FT // P], f32)    # qkv bias, per-partition per f-block
        bvb = const.tile([P, FQ], f32)         # v bias broadcast across partitions

        xT_r = xT.ap().rearrange("(c p) t -> p c t", p=P)
        wT_r = wqkvT.ap().rearrange("(c p) f -> p c f", p=P)
        xs = []
        ws = []
        engs = [nc.sync, nc.scalar, nc.gpsimd]
        for c in range(8):
            xs.append(loadp.tile([P, N], mmdt, name=f"xs{c}"))
            ws.append(loadp.tile([P, FT], mmdt, name=f"ws{c}"))
        # gate-critical set first (everything the first S groups need):
        # all of x plus the pair-0 qk weight slice, round-robined so each
        # queue carries ~the same gate volume
        nc.gpsimd.dma_start(out=bqv, in_=bqkvT.ap())
        rr = 0
        for c in range(8):
            engs[rr % 3].dma_start(out=xs[c], in_=xT_r[:, c])
            rr += 1
            engs[rr % 3].dma_start(
                out=ws[c][:, 0:2 * P], in_=wT_r[:, c, 0:2 * P])
            rr += 1
        # deferred set: remaining qk pairs, v weights, w1, v bias
        for c in range(8):
            engs[rr % 3].dma_start(
                out=ws[c][:, 2 * P:8 * P], in_=wT_r[:, c, 2 * P:8 * P])
            rr += 1
        for c in range(8):
            engs[rr % 3].dma_start(
                out=ws[c][:, 8 * P:12 * P], in_=wT_r[:, c, 8 * P:12 * P])
            rr += 1
        bv_bc = bass.AP(
            tensor=bv.ap().tensor,
            offset=0,
            ap=[[0, P], [1, FQ]],
        )
        nc.gpsimd.dma_start(out=bvb, in_=bv_bc)
        nc.sync.dma_start(out=w1s, in_=w1T.ap().rearrange("(c p) o -> p c o", p=P))
        # ones column of V' (row sums in the PV matmul)
        nc.vector.memset(
            vs.rearrange("p c (h e) -> p c h e", e=65)[:, :, :, 64:65],
            1.0,
        )
        # warmup: force the Exp activation-table DMA (~2.7us) to happen now,
        # while the input DMAs are still in flight
        warm = const.tile([P, 8], f32)
        nc.vector.memset(warm, 0.0)
        nc.scalar.activation(out=warm, in_=warm, func=Exp, scale=1.0)

        def gate_mm(mm, gate):
            if gate is not None:
                add_dep_helper(mm.ins, gate.ins, reason="window balance")

        # ---- QK projection for one head-pair; weight blocks are pair-major
        # on the host: q of pair p at cols [256p, 256p+128), k at +128 ----
        def qk_proj(p_, gate=None):
            for ki in range(2):
                dst = (qt, kt)[ki]
                fb = 2 * p_ + ki
                ps0 = psP.tile([P, TH], f32, tag="pp", name=f"q{fb}a")
                ps1 = psP.tile([P, TH], f32, tag="pp", name=f"q{fb}b")
                pstiles = (ps0, ps1)
                for c in range(8):
                    for th in range(2):
                        mm = nc.tensor.matmul(
                            pstiles[th],
                            ws[c][:, fb * P:(fb + 1) * P],
                            xs[c][:, th * TH:(th + 1) * TH],
                            start=(c == 0),
                            stop=(c == 7),
                        )
                        if c == 0:
                            gate_mm(mm, gate)
                for th in range(2):
                    nc.vector.tensor_scalar_add(
                        out=dst[:, p_, th * TH:(th + 1) * TH],
                        in0=pstiles[th],
                        scalar1=bqv[:, fb:fb + 1],
                    )

        qk_proj(0)

        # ---- V projection (chunk-paced, overlaps the input DMA stream) ----
        def v_proj(jc0, gate=None):
            pv0 = psB.tile([P, TH], f32, tag="pv", name=f"v{jc0}")
            pv1 = psB.tile([P, TH], f32, tag="pv", name=f"v{jc0 + 1}")
            for c in range(8):
                for k, pvt in ((0, pv0), (1, pv1)):
                    mm = nc.tensor.matmul(
                        pvt,
                        xs[c][:, (jc0 + k) * P:(jc0 + k + 1) * P],
                        ws[c][:, 2 * FQ:3 * FQ],
                        start=(c == 0),
                        stop=(c == 7),
                    )
                    if c == 0:
                        gate_mm(mm, gate)
            for k, pvt in ((0, pv0), (1, pv1)):
                jc = jc0 + k
                nc.vector.tensor_add(
                    out=vs[:, jc].rearrange("p (h e) -> p h e", e=65)[:, :, 0:64],
                    in0=pvt.rearrange("p (h e) -> p h e", e=64),
                    in1=bvb.rearrange("p (h e) -> p h e", e=64),
                )

        v_proj(0)
        qk_proj(1)
        for jc0 in range(2, 8, 2):
            v_proj(jc0)

        # ---- attention ----
        with tc.tile_pool(name="ptp", bufs=2) as ptp:
            pts = {}
            exp_insts = {}

            # One fused S-psum tile per (pair, jc): [128, 2048] f32 (4 banks),
            # quadrants (head, ih); ONE exp covers the group.  The 4 S
            # matmuls alternate row tiles (pb 0/64) and issue back-to-back.
            def s_group(p_, jc, pt):
                with tc.high_priority():
                    sf = psS.tile([P, 2048], f32, tag="ps", name=f"s{p_}_{jc}")
                    for ih in range(2):
                        for pbi, pb in ((0, 0), (1, D)):
                            nc.tensor.matmul(
                                sf[:, pbi * 1024 + ih * TH:
                                   pbi * 1024 + (ih + 1) * TH],
                                kt[pb:pb + D, p_, jc * P:(jc + 1) * P],
                                qt[pb:pb + D, p_, ih * TH:(ih + 1) * TH],
                                start=True,
                                stop=True,
                                tile_position=(pb, 0),
                            )
                    return nc.scalar.activation(
                        out=pt[:, jc], in_=sf, func=Exp, scale=1.0 / 32.0,
                    )

            def pair_s_exp(p_):
                pt = ptp.tile([P, 8, 2048], mmdt, tag="pt", name=f"pt{p_}")
                pts[p_] = pt
                for jc in range(8):
                    exp_insts[(p_, jc)] = s_group(p_, jc, pt)

            def pv(h, gate=None):
                hp, hh = divmod(h, 2)
                pt = pts[hp]
                for ih in range(2):
                    ops = psB.tile([P, TH], f32, tag="pv", name=f"o{h}_{ih}")
                    for jc in range(8):
                        mm = nc.tensor.matmul(
                            ops[0:65],
                            vs[:, jc, h * 65:h * 65 + 65],
                            pt[:, jc, hh * 1024 + ih * TH:
                               hh * 1024 + (ih + 1) * TH],
                            start=(jc == 0),
                            stop=(jc == 7),
                        )
                        if jc == 0:
                            gate_mm(mm, gate)
                    norm(h, ih, ops)

            outT_r = outT.ap().rearrange("(b p) t -> p b t", p=P)
            dma_engines = [nc.sync, nc.gpsimd, nc.scalar]

            def final_group(ob, th):
                if th == 1 and ob % 2 == 1:
                    fps = psB.tile([P, TH], f32, tag="pv", name=f"f{ob}_{th}")
                else:
                    fps = psP.tile([P, TH], f32, tag="pp", name=f"f{ob}_{th}")
                for fc in range(4):
                    nc.tensor.matmul(
                        fps,
                        w1s[:, fc, ob * P:(ob + 1) * P],
                        at[:, fc, th * TH:(th + 1) * TH],
                        start=(fc == 0),
                        stop=(fc == 3),
                    )
                ot = outp.tile([P, TH], outdt, tag="ot")
                if th == 0 or ob % 2 == 0:
                    # th=0 wave overlaps the ih=1 exps: keep ACT clear
                    nc.vector.tensor_copy(out=ot, in_=fps)
                else:
                    # ACT is idle once the exps are done
                    nc.scalar.activation(
                        out=ot, in_=fps,
                        func=mybir.ActivationFunctionType.Copy, scale=1.0,
                    )
                if th == 0:
                    # gpsimd is busy with the norm broadcasts here
                    eng = (nc.sync, nc.scalar)[ob % 2]
                else:
                    eng = dma_engines[ob % 3]
                eng.dma_start(
                    out=outT_r[:, ob, th * TH:(th + 1) * TH], in_=ot
                )

            def norm(h, ih, ops):
                hp, hh = divmod(h, 2)
                pb = hh * D
                lrow = small.tile([1, TH], f32, tag="lrow")
                nc.vector.tensor_copy(out=lrow, in_=ops[64:65, :])
                rec = small.tile([1, TH], f32, tag="rec")
                nc.vector.reciprocal_approx_fast(out=rec, in_=lrow)
                bc = small.tile([D, TH], f32, tag="bc")
                nc.gpsimd.partition_broadcast(out_ap=bc, in_ap=rec)
                nc.vector.tensor_mul(
                    out=at[pb:pb + D, hp, ih * TH:(ih + 1) * TH],
                    in0=ops[0:64, :],
                    in1=bc,
                )

            for p_ in range(3):
                pair_s_exp(p_)
                # qk_proj first so the next pair's q/k is never the
                # pair-boundary bottleneck; pv chains fill the rest
                if p_ > 0:
                    qk_proj(p_ + 1, gate=exp_insts[(p_, 0)])
                if p_ > 0:
                    gate = exp_insts[(p_, 0)]
                    pv(2 * p_ - 2, gate=gate)
                    pv(2 * p_ - 1, gate=gate)
            pair3_gate = exp_insts[(2, 7)]
            pv(4, gate=pair3_gate)
            pv(5, gate=pair3_gate)
            # ---- last pair, ih-major with per-ih exps: the ih=0 half of
            # the output projection (th=0 wave, psP) overlaps the ih=1
            # attention half.  Both PV psums live on psB so psP stays free
            # for the wave. ----
            pt3 = ptp.tile([P, 8, 2048], mmdt, tag="pt", name="pt3")
            pts[3] = pt3
            for ih in range(2):
                o6i = psB.tile([P, TH], f32, tag="pv", name=f"o6_{ih}")
                o7i = psB.tile([P, TH], f32, tag="pv", name=f"o7_{ih}")
                for jc in range(8):
                    with tc.high_priority():
                        sf = psS.tile([P, 1024], f32, tag="ps",
                                      name=f"s3_{jc}_{ih}")
                        for pbi, pb in ((0, 0), (1, D)):
                            nc.tensor.matmul(
                                sf[:, pbi * TH:(pbi + 1) * TH],
                                kt[pb:pb + D, 3, jc * P:(jc + 1) * P],
                                qt[pb:pb + D, 3, ih * TH:(ih + 1) * TH],
                                start=True,
                                stop=True,
                                tile_position=(pb, 0),
                            )
                        nc.scalar.activation(
                            out=pt3[:, jc].rearrange(
                                "p (hh x) -> p hh x", hh=2)
                            [:, :, ih * TH:(ih + 1) * TH],
                            in_=sf, func=Exp, scale=1.0 / 32.0,
                        )
                    nc.tensor.matmul(
                        o6i[0:65],
                        vs[:, jc, 6 * 65:6 * 65 + 65],
                        pt3[:, jc, ih * TH:(ih + 1) * TH],
                        start=(jc == 0),
                        stop=(jc == 7),
                    )
                    nc.tensor.matmul(
                        o7i[0:65],
                        vs[:, jc, 7 * 65:7 * 65 + 65],
                        pt3[:, jc, 1024 + ih * TH: 1024 + (ih + 1) * TH],
                        start=(jc == 0),
                        stop=(jc == 7),
                    )
                norm(6, ih, o6i)
                norm(7, ih, o7i)
                # out-proj wave for this token half (all on psP)
                for ob in range(8):
                    final_group(ob, ih)

    nc.finalize()
    return nc


def _get_nc(mm_dtype=None):
    if mm_dtype is None:
        mm_dtype = MM_DTYPE
    if mm_dtype not in _CACHE:
        _CACHE[mm_dtype] = _build(mm_dtype)
    return _CACHE[mm_dtype]


def make_in_maps(x, Wqkv, bqkv, W1):
    import ml_dtypes
    mmnp = ml_dtypes.bfloat16 if MM_DTYPE == "bfloat16" else np.float32
    x = np.ascontiguousarray(np.asarray(x, dtype=np.float32))
    Wqkv = np.asarray(Wqkv, dtype=np.float32)
    bqkv = np.asarray(bqkv, dtype=np.float32)
    W1 = np.asarray(W1, dtype=np.float32)
    in_maps = []
    for c in range(NCORES):
        b, hg = divmod(c, HG)
        qsl = slice(hg * FQ, (hg + 1) * FQ)
        ksl = slice(DIM + hg * FQ, DIM + (hg + 1) * FQ)
        vsl = slice(2 * DIM + hg * FQ, 2 * DIM + (hg + 1) * FQ)
        q, k, v = Wqkv[qsl], Wqkv[ksl], Wqkv[vsl]
        bq, bk, bvv = bqkv[qsl], bqkv[ksl], bqkv[vsl]
        # pair-major qk interleave: [q_p0, k_p0, q_p1, k_p1, ..., v]
        blocks = []
        bblocks = []
        for p in range(4):
            blocks.append(q[p * P:(p + 1) * P])
            blocks.append(k[p * P:(p + 1) * P])
            bblocks.append(bq[p * P:(p + 1) * P])
            bblocks.append(bk[p * P:(p + 1) * P])
        blocks.append(v)
        bblocks.append(bvv)
        w_loc = np.concatenate(blocks, axis=0)
        b_loc = np.concatenate(bblocks)
        in_maps.append({
            "xT": np.ascontiguousarray(x[b].T.astype(mmnp)),
            "wqkvT": np.ascontiguousarray(w_loc.T.astype(mmnp)),
            "bqkvT": np.ascontiguousarray(b_loc.reshape(FT // P, P).T),
            "bv": np.ascontiguousarray(bqkv[vsl]),
            "w1T": np.ascontiguousarray(W1[:, hg * FQ:(hg + 1) * FQ].T.astype(mmnp)),
        })
    return in_maps


def combine_outputs(results, b1):
    b1 = np.asarray(b1, dtype=np.float32)
    out = np.empty((B, N, DIM), dtype=np.float32)
    for b in range(B):
        acc = (results[HG * b]["outT"].astype(np.float32)
               + results[HG * b + 1]["outT"].astype(np.float32))
        out[b] = acc.T + b1
    return out


def kernel(x, Wqkv, bqkv, W1, b1, trace=False):
    from concourse.bass_utils import run_bass_kernel_spmd

    nc = _get_nc()
    in_maps = make_in_maps(x, Wqkv, bqkv, W1)
    res = run_bass_kernel_spmd(
        nc, in_maps, core_ids=list(range(NCORES)), trace=trace
    )
    out = combine_outputs(res.results, b1)
    if trace:
        kernel.last_result = res
    return out


# revision 29
# speedup vs baseline: 1.1360x; 1.1360x over previous
"""Multi-head attention block (QKV proj + softmax attention + out proj) on 8
Trainium2 NeuronCores.

Problem shapes: x [4, 1024, 1024], Wqkv [3072, 1024], bqkv [3072],
W1 [1024, 1024], b1 [1024].  out = Attention(x) @ W1.T + b1, 16 heads, d=64,
softmax scale 1/sqrt(1024) = 1/32.

Sharding: core c handles batch b = c // 2 and head-group hg = c % 2 (8 of the
16 heads).  Each core computes its heads' QKV projection, full attention for
those heads over its batch, and a *partial* output projection against the
W1 columns its heads feed.  The host sums the two partials per batch and adds
b1.  No device collectives.

Structure (v7):
  - S psum fused per (pair, jc) into one [128, 2048] tile (4 banks) covering
    both heads x both query halves; ONE exp per group amortizes the ACT
    PSUM-read bubble and halves the number of chain round-trips; the 4 S
    matmuls issue back-to-back so the row-tiled (tile_position 0/64) pairs
    run concurrently on the PE.
  - S+exp emitted under tc.high_priority() so the static scheduler never
    parks filler matmuls ahead of the pacing chain (PE FIFO is strict).
  - Window balancing: qk_proj(2)/qk_proj(3)/pv chains are dep-gated to the
    attention window where their PE time is needed; otherwise the scheduler
    drains all projection work early and the PE starves (and HAM-cools) in
    the later windows.
  - Host reorders Wqkv columns pair-major ([q0 k0 q1 k1 ... v]) so the
    pair-0 slice is one contiguous early DMA; input DMAs are split into
    gate-critical (x, qk pair-0) vs deferred (rest) sets balanced over the
    3 DMA queues, which pulls the first exp ~13us earlier.
  - Tail: all 16 output-projection groups on psP, psum->bf16 casts split
    DVE/ACT, output DMAs rotated over the 3 queues.
"""

import numpy as np

B = 4
N = 1024            # tokens per batch
DIM = 1024          # model dim
HEADS = 16
D = DIM // HEADS    # 64
NCORES = 8
HG = 2              # head groups (tensor-parallel degree over heads)
NHL = HEADS // HG   # 8 local heads
FQ = NHL * D        # 512 local q (or k or v) features
FT = 3 * FQ         # 1536 local qkv features
P = 128
TH = 512            # token half (matmul free dim)

_CACHE = {}
MM_DTYPE = "bfloat16"


def _build(mm_dtype=None):
    if mm_dtype is None:
        mm_dtype = MM_DTYPE
    from contextlib import ExitStack

    import concourse.bacc as bacc
    import concourse.bass as bass
    import concourse.tile as tile
    from concourse import mybir
    from concourse.tile import add_dep_helper

    f32 = mybir.dt.float32
    mmdt = getattr(mybir.dt, mm_dtype)

    nc = bacc.Bacc("TRN2", target_bir_lowering=False)

    xT = nc.dram_tensor("xT", [DIM, N], mmdt, kind="ExternalInput")
    wqkvT = nc.dram_tensor("wqkvT", [DIM, FT], mmdt, kind="ExternalInput")
    bqkvT = nc.dram_tensor("bqkvT", [P, FT // P], f32, kind="ExternalInput")
    bv = nc.dram_tensor("bv", [FQ], f32, kind="ExternalInput")
    w1T = nc.dram_tensor("w1T", [FQ, DIM], mmdt, kind="ExternalInput")
    outdt = mmdt if mm_dtype == "bfloat16" else f32
    outT = nc.dram_tensor("outT", [DIM, N], outdt, kind="ExternalOutput")

    Exp = mybir.ActivationFunctionType.Exp

    with tile.TileContext(nc) as tc, ExitStack() as ctx:
        const = ctx.enter_context(tc.tile_pool(name="const", bufs=1))
        psS = ctx.enter_context(tc.tile_pool(name="psS", bufs=1, space="PSUM"))
        psP = ctx.enter_context(tc.tile_pool(name="psP", bufs=2, space="PSUM"))
        psB = ctx.enter_context(tc.tile_pool(name="psB", bufs=2, space="PSUM"))
        outp = ctx.enter_context(tc.tile_pool(name="outp", bufs=6))
        small = ctx.enter_context(tc.tile_pool(name="small", bufs=4))
        loadp = ctx.enter_context(tc.tile_pool(name="loadp", bufs=1))

        # persistent SBUF
        qt = const.tile([P, 4, N], mmdt)        # Q.T  [f-inner, head-pair, tok]
        kt = const.tile([P, 4, N], mmdt)        # K.T
        vs = const.tile([P, 8, NHL * 65], mmdt)  # V'  [tok-inner, j-chunk, h*65+e]
        at = const.tile([P, 4, N], mmdt)        # A.T  [f-inner, f-chunk, tok]
        w1s = const.tile([P, 4, DIM], mmdt)     # W1loc.T [f-inner, f-chunk, out]
        bqv = const.tile([P, FT // P], f32)    # qkv bias, per-partition per f-block
        bvb = const.tile([P, FQ], f32)         # v bias broadcast across partitions

        xT_r = xT.ap().rearrange("(c p) t -> p c t", p=P)
        wT_r = wqkvT.ap().rearrange("(c p) f -> p c f", p=P)
        xs = []
        ws = []
        engs = [nc.sync, nc.scalar, nc.gpsimd]
        for c in range(8):
            xs.append(loadp.tile([P, N], mmdt, name=f"xs{c}"))
            ws.append(loadp.tile([P, FT], mmdt, name=f"ws{c}"))
        # gate-critical set first (everything the first S groups need):
        # all of x plus the pair-0 qk weight slice, round-robined so each
        # queue carries ~the same gate volume
        nc.gpsimd.dma_start(out=bqv, in_=bqkvT.ap())
        rr = 0
        for c in range(8):
            engs[rr % 3].dma_start(out=xs[c], in_=xT_r[:, c])
            rr += 1
            engs[rr % 3].dma_start(
                out=ws[c][:, 0:2 * P], in_=wT_r[:, c, 0:2 * P])
            rr += 1
        # deferred set: remaining qk pairs, v weights, w1, v bias
        for c in range(8):
            engs[rr % 3].dma_start(
                out=ws[c][:, 2 * P:8 * P], in_=wT_r[:, c, 2 * P:8 * P])
            rr += 1
        for c in range(8):
            engs[rr % 3].dma_start(
                out=ws[c][:, 8 * P:12 * P], in_=wT_r[:, c, 8 * P:12 * P])
            rr += 1
        bv_bc = bass.AP(
            tensor=bv.ap().tensor,
            offset=0,
            ap=[[0, P], [1, FQ]],
        )
        nc.gpsimd.dma_start(out=bvb, in_=bv_bc)
        nc.sync.dma_start(out=w1s, in_=w1T.ap().rearrange("(c p) o -> p c o", p=P))
        # ones column of V' (row sums in the PV matmul)
        nc.vector.memset(
            vs.rearrange("p c (h e) -> p c h e", e=65)[:, :, :, 64:65],
            1.0,
        )
        # warmup: force the Exp activation-table DMA (~2.7us) to happen now,
        # while the input DMAs are still in flight
        warm = const.tile([P, 8], f32)
        nc.vector.memset(warm, 0.0)
        nc.scalar.activation(out=warm, in_=warm, func=Exp, scale=1.0)

        def gate_mm(mm, gate):
            if gate is not None:
                add_dep_helper(mm.ins, gate.ins, reason="window balance")

        # ---- QK projection for one head-pair; weight blocks are pair-major
        # on the host: q of pair p at cols [256p, 256p+128), k at +128 ----
        def qk_proj(p_, gate=None):
            for ki in range(2):
                dst = (qt, kt)[ki]
                fb = 2 * p_ + ki
                ps0 = psP.tile([P, TH], f32, tag="pp", name=f"q{fb}a")
                ps1 = psP.tile([P, TH], f32, tag="pp", name=f"q{fb}b")
                pstiles = (ps0, ps1)
                for c in range(8):
                    for th in range(2):
                        mm = nc.tensor.matmul(
                            pstiles[th],
                            ws[c][:, fb * P:(fb + 1) * P],
                            xs[c][:, th * TH:(th + 1) * TH],
                            start=(c == 0),
                            stop=(c == 7),
                        )
                        if c == 0:
                            gate_mm(mm, gate)
                for th in range(2):
                    nc.vector.tensor_scalar_add(
                        out=dst[:, p_, th * TH:(th + 1) * TH],
                        in0=pstiles[th],
                        scalar1=bqv[:, fb:fb + 1],
                    )

        qk_proj(0)

        # ---- V projection (chunk-paced, overlaps the input DMA stream) ----
        def v_proj(jc0, gate=None):
            pv0 = psB.tile([P, TH], f32, tag="pv", name=f"v{jc0}")
            pv1 = psB.tile([P, TH], f32, tag="pv", name=f"v{jc0 + 1}")
            for c in range(8):
                for k, pvt in ((0, pv0), (1, pv1)):
                    mm = nc.tensor.matmul(
                        pvt,
                        xs[c][:, (jc0 + k) * P:(jc0 + k + 1) * P],
                        ws[c][:, 2 * FQ:3 * FQ],
                        start=(c == 0),
                        stop=(c == 7),
                    )
                    if c == 0:
                        gate_mm(mm, gate)
            for k, pvt in ((0, pv0), (1, pv1)):
                jc = jc0 + k
                nc.vector.tensor_add(
                    out=vs[:, jc].rearrange("p (h e) -> p h e", e=65)[:, :, 0:64],
                    in0=pvt.rearrange("p (h e) -> p h e", e=64),
                    in1=bvb.rearrange("p (h e) -> p h e", e=64),
                )

        v_proj(0)
        qk_proj(1)
        for jc0 in range(2, 8, 2):
            v_proj(jc0)

        # ---- attention ----
        with tc.tile_pool(name="ptp", bufs=2) as ptp:
            pts = {}
            exp_insts = {}

            # One fused S-psum tile per (pair, jc): [128, 2048] f32 (4 banks),
            # quadrants (head, ih); ONE exp covers the group.  The 4 S
            # matmuls alternate row tiles (pb 0/64) and issue back-to-back.
            def s_group(p_, jc, pt):
                with tc.high_priority():
                    sf = psS.tile([P, 2048], f32, tag="ps", name=f"s{p_}_{jc}")
                    for ih in range(2):
                        for pbi, pb in ((0, 0), (1, D)):
                            nc.tensor.matmul(
                                sf[:, pbi * 1024 + ih * TH:
                                   pbi * 1024 + (ih + 1) * TH],
                                kt[pb:pb + D, p_, jc * P:(jc + 1) * P],
                                qt[pb:pb + D, p_, ih * TH:(ih + 1) * TH],
                                start=True,
                                stop=True,
                                tile_position=(pb, 0),
                            )
                    return nc.scalar.activation(
                        out=pt[:, jc], in_=sf, func=Exp, scale=1.0 / 32.0,
                    )

            def pair_s_exp(p_):
                pt = ptp.tile([P, 8, 2048], mmdt, tag="pt", name=f"pt{p_}")
                pts[p_] = pt
                for jc in range(8):
                    exp_insts[(p_, jc)] = s_group(p_, jc, pt)

            def pv(h, gate=None):
                hp, hh = divmod(h, 2)
                pt = pts[hp]
                for ih in range(2):
                    ops = psB.tile([P, TH], f32, tag="pv", name=f"o{h}_{ih}")
                    for jc in range(8):
                        mm = nc.tensor.matmul(
                            ops[0:65],
                            vs[:, jc, h * 65:h * 65 + 65],
                            pt[:, jc, hh * 1024 + ih * TH:
                               hh * 1024 + (ih + 1) * TH],
                            start=(jc == 0),
                            stop=(jc == 7),
                        )
                        if jc == 0:
                            gate_mm(mm, gate)
                    norm(h, ih, ops)

            outT_r = outT.ap().rearrange("(b p) t -> p b t", p=P)
            dma_engines = [nc.sync, nc.gpsimd, nc.scalar]

            def final_group(ob, th):
                fps = psP.tile([P, TH], f32, tag="pp", name=f"f{ob}_{th}")
                for fc in range(4):
                    nc.tensor.matmul(
                        fps,
                        w1s[:, fc, ob * P:(ob + 1) * P],
                        at[:, fc, th * TH:(th + 1) * TH],
                        start=(fc == 0),
                        stop=(fc == 3),
                    )
                ot = outp.tile([P, TH], outdt, tag="ot")
                if th == 0 or ob % 2 == 0:
                    # th=0 wave overlaps the ih=1 exps: keep ACT clear
                    nc.vector.tensor_copy(out=ot, in_=fps)
                else:
                    # ACT is idle once the exps are done
                    nc.scalar.activation(
                        out=ot, in_=fps,
                        func=mybir.ActivationFunctionType.Copy, scale=1.0,
                    )
                if th == 0:
                    # gpsimd is busy with the norm broadcasts here
                    eng = (nc.sync, nc.scalar)[ob % 2]
                else:
                    eng = dma_engines[ob % 3]
                eng.dma_start(
                    out=outT_r[:, ob, th * TH:(th + 1) * TH], in_=ot
                )

            def norm(h, ih, ops):
                hp, hh = divmod(h, 2)
                pb = hh * D
                lrow = small.tile([1, TH], f32, tag="lrow")
                nc.vector.tensor_copy(out=lrow, in_=ops[64:65, :])
                rec = small.tile([1, TH], f32, tag="rec")
                nc.vector.reciprocal_approx_fast(out=rec, in_=lrow)
                bc = small.tile([D, TH], f32, tag="bc")
                nc.gpsimd.partition_broadcast(out_ap=bc, in_ap=rec)
                nc.vector.tensor_mul(
                    out=at[pb:pb + D, hp, ih * TH:(ih + 1) * TH],
                    in0=ops[0:64, :],
                    in1=bc,
                )

            for p_ in range(3):
                pair_s_exp(p_)
                # qk_proj first so the next pair's q/k is never the
                # pair-boundary bottleneck; pv chains fill the rest
                if p_ > 0:
                    qk_proj(p_ + 1, gate=exp_insts[(p_, 0)])
                if p_ > 0:
                    gate = exp_insts[(p_, 0)]
                    pv(2 * p_ - 2, gate=gate)
                    pv(2 * p_ - 1, gate=gate)
            pair3_gate = exp_insts[(2, 7)]
            pv(4, gate=pair3_gate)
            pv(5, gate=pair3_gate)
            # ---- last pair, ih-major with per-ih exps: the ih=0 half of
            # the output projection (th=0 wave, psP) overlaps the ih=1
            # attention half.  Both PV psums live on psB so psP stays free
            # for the wave. ----
            pt3 = ptp.tile([P, 8, 2048], mmdt, tag="pt", name="pt3")
            pts[3] = pt3
            for ih in range(2):
                o6i = psB.tile([P, TH], f32, tag="pv", name=f"o6_{ih}")
                o7i = psB.tile([P, TH], f32, tag="pv", name=f"o7_{ih}")
                for jc in range(8):
                    with tc.high_priority():
                        sf = psS.tile([P, 1024], f32, tag="ps",
                                      name=f"s3_{jc}_{ih}")
                        for pbi, pb in ((0, 0), (1, D)):
                            nc.tensor.matmul(
                                sf[:, pbi * TH:(pbi + 1) * TH],
                                kt[pb:pb + D, 3, jc * P:(jc + 1) * P],
                                qt[pb:pb + D, 3, ih * TH:(ih + 1) * TH],
                                start=True,
                                stop=True,
                                tile_position=(pb, 0),
                            )
                        nc.scalar.activation(
                            out=pt3[:, jc].rearrange(
                                "p (hh x) -> p hh x", hh=2)
                            [:, :, ih * TH:(ih + 1) * TH],
                            in_=sf, func=Exp, scale=1.0 / 32.0,
                        )
                    nc.tensor.matmul(
                        o6i[0:65],
                        vs[:, jc, 6 * 65:6 * 65 + 65],
                        pt3[:, jc, ih * TH:(ih + 1) * TH],
                        start=(jc == 0),
                        stop=(jc == 7),
                    )
                    nc.tensor.matmul(
                        o7i[0:65],
                        vs[:, jc, 7 * 65:7 * 65 + 65],
                        pt3[:, jc, 1024 + ih * TH: 1024 + (ih + 1) * TH],
                        start=(jc == 0),
                        stop=(jc == 7),
                    )
                norm(6, ih, o6i)
                norm(7, ih, o7i)
                # out-proj wave for this token half (all on psP)
                for ob in range(8):
                    final_group(ob, ih)

    nc.finalize()
    return nc


def _get_nc(mm_dtype=None):
    if mm_dtype is None:
        mm_dtype = MM_DTYPE
    if mm_dtype not in _CACHE:
        _CACHE[mm_dtype] = _build(mm_dtype)
    return _CACHE[mm_dtype]


def make_in_maps(x, Wqkv, bqkv, W1):
    import ml_dtypes
    mmnp = ml_dtypes.bfloat16 if MM_DTYPE == "bfloat16" else np.float32
    x = np.ascontiguousarray(np.asarray(x, dtype=np.float32))
    Wqkv = np.asarray(Wqkv, dtype=np.float32)
    bqkv = np.asarray(bqkv, dtype=np.float32)
    W1 = np.asarray(W1, dtype=np.float32)
    in_maps = []
    for c in range(NCORES):
        b, hg = divmod(c, HG)
        qsl = slice(hg * FQ, (hg + 1) * FQ)
        ksl = slice(DIM + hg * FQ, DIM + (hg + 1) * FQ)
        vsl = slice(2 * DIM + hg * FQ, 2 * DIM + (hg + 1) * FQ)
        q, k, v = Wqkv[qsl], Wqkv[ksl], Wqkv[vsl]
        bq, bk, bvv = bqkv[qsl], bqkv[ksl], bqkv[vsl]
        # pair-major qk interleave: [q_p0, k_p0, q_p1, k_p1, ..., v]
        blocks = []
        bblocks = []
        for p in range(4):
            blocks.append(q[p * P:(p + 1) * P])
            blocks.append(k[p * P:(p + 1) * P])
            bblocks.append(bq[p * P:(p + 1) * P])
            bblocks.append(bk[p * P:(p + 1) * P])
        blocks.append(v)
        bblocks.append(bvv)
        w_loc = np.concatenate(blocks, axis=0)
        b_loc = np.concatenate(bblocks)
        in_maps.append({
            "xT": np.ascontiguousarray(x[b].T.astype(mmnp)),
            "wqkvT": np.ascontiguousarray(w_loc.T.astype(mmnp)),
            "bqkvT": np.ascontiguousarray(b_loc.reshape(FT // P, P).T),
            "bv": np.ascontiguousarray(bqkv[vsl]),
            "w1T": np.ascontiguousarray(W1[:, hg * FQ:(hg + 1) * FQ].T.astype(mmnp)),
        })
    return in_maps


def combine_outputs(results, b1):
    b1 = np.asarray(b1, dtype=np.float32)
    out = np.empty((B, N, DIM), dtype=np.float32)
    for b in range(B):
        acc = (results[HG * b]["outT"].astype(np.float32)
               + results[HG * b + 1]["outT"].astype(np.float32))
        out[b] = acc.T + b1
    return out


def kernel(x, Wqkv, bqkv, W1, b1, trace=False):
    from concourse.bass_utils import run_bass_kernel_spmd

    nc = _get_nc()
    in_maps = make_in_maps(x, Wqkv, bqkv, W1)
    res = run_bass_kernel_spmd(
        nc, in_maps, core_ids=list(range(NCORES)), trace=trace
    )
    out = combine_outputs(res.results, b1)
    if trace:
        kernel.last_result = res
    return out


# revision 30
# speedup vs baseline: 1.1784x; 1.0374x over previous
"""Multi-head attention block (QKV proj + softmax attention + out proj) on 8
Trainium2 NeuronCores.

Problem shapes: x [4, 1024, 1024], Wqkv [3072, 1024], bqkv [3072],
W1 [1024, 1024], b1 [1024].  out = Attention(x) @ W1.T + b1, 16 heads, d=64,
softmax scale 1/sqrt(1024) = 1/32.

Sharding: core c handles batch b = c // 2 and head-group hg = c % 2 (8 of the
16 heads).  Each core computes its heads' QKV projection, full attention for
those heads over its batch, and a *partial* output projection against the
W1 columns its heads feed.  The host sums the two partials per batch and adds
b1.  No device collectives.

Structure (v7):
  - S psum fused per (pair, jc) into one [128, 2048] tile (4 banks) covering
    both heads x both query halves; ONE exp per group amortizes the ACT
    PSUM-read bubble and halves the number of chain round-trips; the 4 S
    matmuls issue back-to-back so the row-tiled (tile_position 0/64) pairs
    run concurrently on the PE.
  - S+exp emitted under tc.high_priority() so the static scheduler never
    parks filler matmuls ahead of the pacing chain (PE FIFO is strict).
  - Window balancing: qk_proj(2)/qk_proj(3)/pv chains are dep-gated to the
    attention window where their PE time is needed; otherwise the scheduler
    drains all projection work early and the PE starves (and HAM-cools) in
    the later windows.
  - Host reorders Wqkv columns pair-major ([q0 k0 q1 k1 ... v]) so the
    pair-0 slice is one contiguous early DMA; input DMAs are split into
    gate-critical (x, qk pair-0) vs deferred (rest) sets balanced over the
    3 DMA queues, which pulls the first exp ~13us earlier.
  - Tail: all 16 output-projection groups on psP, psum->bf16 casts split
    DVE/ACT, output DMAs rotated over the 3 queues.
"""

import numpy as np

B = 4
N = 1024            # tokens per batch
DIM = 1024          # model dim
HEADS = 16
D = DIM // HEADS    # 64
NCORES = 8
HG = 2              # head groups (tensor-parallel degree over heads)
NHL = HEADS // HG   # 8 local heads
FQ = NHL * D        # 512 local q (or k or v) features
FT = 3 * FQ         # 1536 local qkv features
P = 128
TH = 512            # token half (matmul free dim)

_CACHE = {}
MM_DTYPE = "bfloat16"


def _build(mm_dtype=None):
    if mm_dtype is None:
        mm_dtype = MM_DTYPE
    from contextlib import ExitStack

    import concourse.bacc as bacc
    import concourse.bass as bass
    import concourse.tile as tile
    from concourse import mybir
    from concourse.tile import add_dep_helper

    f32 = mybir.dt.float32
    mmdt = getattr(mybir.dt, mm_dtype)

    nc = bacc.Bacc("TRN2", target_bir_lowering=False)

    xT = nc.dram_tensor("xT", [DIM, N], mmdt, kind="ExternalInput")
    wqkvT = nc.dram_tensor("wqkvT", [DIM, FT], mmdt, kind="ExternalInput")
    bqkvT = nc.dram_tensor("bqkvT", [P, FT // P], f32, kind="ExternalInput")
    bv = nc.dram_tensor("bv", [FQ], f32, kind="ExternalInput")
    w1T = nc.dram_tensor("w1T", [FQ, DIM], mmdt, kind="ExternalInput")
    outdt = mmdt if mm_dtype == "bfloat16" else f32
    outT = nc.dram_tensor("outT", [DIM, N], outdt, kind="ExternalOutput")

    Exp = mybir.ActivationFunctionType.Exp

    with tile.TileContext(nc) as tc, ExitStack() as ctx:
        const = ctx.enter_context(tc.tile_pool(name="const", bufs=1))
        psS = ctx.enter_context(tc.tile_pool(name="psS", bufs=1, space="PSUM"))
        psP = ctx.enter_context(tc.tile_pool(name="psP", bufs=2, space="PSUM"))
        psB = ctx.enter_context(tc.tile_pool(name="psB", bufs=2, space="PSUM"))
        outp = ctx.enter_context(tc.tile_pool(name="outp", bufs=6))
        small = ctx.enter_context(tc.tile_pool(name="small", bufs=4))
        loadp = ctx.enter_context(tc.tile_pool(name="loadp", bufs=1))

        # persistent SBUF
        qt = const.tile([P, 4, N], mmdt)        # Q.T  [f-inner, head-pair, tok]
        kt = const.tile([P, 4, N], mmdt)        # K.T
        vs = const.tile([P, 8, NHL * 65], mmdt)  # V'  [tok-inner, j-chunk, h*65+e]
        at = const.tile([P, 4, N], mmdt)        # A.T  [f-inner, f-chunk, tok]
        w1s = const.tile([P, 4, DIM], mmdt)     # W1loc.T [f-inner, f-chunk, out]
        bqv = const.tile([P, FT // P], f32)    # qkv bias, per-partition per f-block
        bvb = const.tile([P, FQ], f32)         # v bias broadcast across partitions

        xT_r = xT.ap().rearrange("(c p) t -> p c t", p=P)
        wT_r = wqkvT.ap().rearrange("(c p) f -> p c f", p=P)
        xs = []
        ws = []
        engs = [nc.sync, nc.scalar, nc.gpsimd]
        for c in range(8):
            xs.append(loadp.tile([P, N], mmdt, name=f"xs{c}"))
            ws.append(loadp.tile([P, FT], mmdt, name=f"ws{c}"))
        # gate-critical set first (everything the first S groups need):
        # all of x plus the pair-0 qk weight slice, round-robined so each
        # queue carries ~the same gate volume
        nc.gpsimd.dma_start(out=bqv, in_=bqkvT.ap())
        rr = 0
        for c in range(8):
            engs[rr % 3].dma_start(out=xs[c], in_=xT_r[:, c])
            rr += 1
            engs[rr % 3].dma_start(
                out=ws[c][:, 0:2 * P], in_=wT_r[:, c, 0:2 * P])
            rr += 1
        # deferred set: remaining qk pairs, v weights, w1, v bias
        for c in range(8):
            engs[rr % 3].dma_start(
                out=ws[c][:, 2 * P:8 * P], in_=wT_r[:, c, 2 * P:8 * P])
            rr += 1
        for c in range(8):
            engs[rr % 3].dma_start(
                out=ws[c][:, 8 * P:12 * P], in_=wT_r[:, c, 8 * P:12 * P])
            rr += 1
        bv_bc = bass.AP(
            tensor=bv.ap().tensor,
            offset=0,
            ap=[[0, P], [1, FQ]],
        )
        nc.gpsimd.dma_start(out=bvb, in_=bv_bc)
        nc.sync.dma_start(out=w1s, in_=w1T.ap().rearrange("(c p) o -> p c o", p=P))
        # ones column of V' (row sums in the PV matmul)
        nc.vector.memset(
            vs.rearrange("p c (h e) -> p c h e", e=65)[:, :, :, 64:65],
            1.0,
        )
        # warmup: force the Exp activation-table DMA (~2.7us) to happen now,
        # while the input DMAs are still in flight
        warm = const.tile([P, 8], f32)
        nc.vector.memset(warm, 0.0)
        nc.scalar.activation(out=warm, in_=warm, func=Exp, scale=1.0)

        def gate_mm(mm, gate):
            if gate is not None:
                add_dep_helper(mm.ins, gate.ins, reason="window balance")

        # ---- QK projection for one head-pair; weight blocks are pair-major
        # on the host: q of pair p at cols [256p, 256p+128), k at +128 ----
        def qk_proj(p_, gate=None):
            for ki in range(2):
                dst = (qt, kt)[ki]
                fb = 2 * p_ + ki
                ps0 = psP.tile([P, TH], f32, tag="pp", name=f"q{fb}a")
                ps1 = psP.tile([P, TH], f32, tag="pp", name=f"q{fb}b")
                pstiles = (ps0, ps1)
                for c in range(8):
                    for th in range(2):
                        mm = nc.tensor.matmul(
                            pstiles[th],
                            ws[c][:, fb * P:(fb + 1) * P],
                            xs[c][:, th * TH:(th + 1) * TH],
                            start=(c == 0),
                            stop=(c == 7),
                        )
                        if c == 0:
                            gate_mm(mm, gate)
                for th in range(2):
                    nc.vector.tensor_scalar_add(
                        out=dst[:, p_, th * TH:(th + 1) * TH],
                        in0=pstiles[th],
                        scalar1=bqv[:, fb:fb + 1],
                    )

        qk_proj(0)

        # ---- V projection (chunk-paced, overlaps the input DMA stream) ----
        def v_proj(jc0, gate=None):
            pv0 = psB.tile([P, TH], f32, tag="pv", name=f"v{jc0}")
            pv1 = psB.tile([P, TH], f32, tag="pv", name=f"v{jc0 + 1}")
            for c in range(8):
                for k, pvt in ((0, pv0), (1, pv1)):
                    mm = nc.tensor.matmul(
                        pvt,
                        xs[c][:, (jc0 + k) * P:(jc0 + k + 1) * P],
                        ws[c][:, 2 * FQ:3 * FQ],
                        start=(c == 0),
                        stop=(c == 7),
                    )
                    if c == 0:
                        gate_mm(mm, gate)
            for k, pvt in ((0, pv0), (1, pv1)):
                jc = jc0 + k
                nc.vector.tensor_add(
                    out=vs[:, jc].rearrange("p (h e) -> p h e", e=65)[:, :, 0:64],
                    in0=pvt.rearrange("p (h e) -> p h e", e=64),
                    in1=bvb.rearrange("p (h e) -> p h e", e=64),
                )

        v_proj(0)
        qk_proj(1)
        for jc0 in range(2, 8, 2):
            v_proj(jc0)

        # ---- attention ----
        with tc.tile_pool(name="ptp", bufs=2) as ptp:
            pts = {}
            exp_insts = {}

            # One fused S-psum tile per (pair, jc): [128, 2048] f32 (4 banks),
            # quadrants (head, ih); ONE exp covers the group.  The 4 S
            # matmuls alternate row tiles (pb 0/64) and issue back-to-back.
            def s_group(p_, jc, pt):
                with tc.high_priority():
                    sf = psS.tile([P, 2048], f32, tag="ps", name=f"s{p_}_{jc}")
                    for ih in range(2):
                        for pbi, pb in ((0, 0), (1, D)):
                            nc.tensor.matmul(
                                sf[:, pbi * 1024 + ih * TH:
                                   pbi * 1024 + (ih + 1) * TH],
                                kt[pb:pb + D, p_, jc * P:(jc + 1) * P],
                                qt[pb:pb + D, p_, ih * TH:(ih + 1) * TH],
                                start=True,
                                stop=True,
                                tile_position=(pb, 0),
                            )
                    return nc.scalar.activation(
                        out=pt[:, jc], in_=sf, func=Exp, scale=1.0 / 32.0,
                    )

            def pair_s_exp(p_):
                pt = ptp.tile([P, 8, 2048], mmdt, tag="pt", name=f"pt{p_}")
                pts[p_] = pt
                for jc in range(8):
                    exp_insts[(p_, jc)] = s_group(p_, jc, pt)

            def pv(h, gate=None):
                hp, hh = divmod(h, 2)
                pt = pts[hp]
                for ih in range(2):
                    ops = psB.tile([P, TH], f32, tag="pv", name=f"o{h}_{ih}")
                    for jc in range(8):
                        mm = nc.tensor.matmul(
                            ops[0:65],
                            vs[:, jc, h * 65:h * 65 + 65],
                            pt[:, jc, hh * 1024 + ih * TH:
                               hh * 1024 + (ih + 1) * TH],
                            start=(jc == 0),
                            stop=(jc == 7),
                        )
                        if jc == 0:
                            gate_mm(mm, gate)
                    norm(h, ih, ops)

            outT_r = outT.ap().rearrange("(b p) t -> p b t", p=P)
            dma_engines = [nc.sync, nc.gpsimd, nc.scalar]

            def final_group(ob, th):
                if th == 1 and ob % 2 == 1:
                    fps = psB.tile([P, TH], f32, tag="pv", name=f"f{ob}_{th}")
                else:
                    fps = psP.tile([P, TH], f32, tag="pp", name=f"f{ob}_{th}")
                for fc in range(4):
                    nc.tensor.matmul(
                        fps,
                        w1s[:, fc, ob * P:(ob + 1) * P],
                        at[:, fc, th * TH:(th + 1) * TH],
                        start=(fc == 0),
                        stop=(fc == 3),
                    )
                ot = outp.tile([P, TH], outdt, tag="ot")
                if th == 0 or ob % 2 == 0:
                    # th=0 wave overlaps the ih=1 exps: keep ACT clear
                    nc.vector.tensor_copy(out=ot, in_=fps)
                else:
                    # ACT is idle once the exps are done
                    nc.scalar.activation(
                        out=ot, in_=fps,
                        func=mybir.ActivationFunctionType.Copy, scale=1.0,
                    )
                if th == 0:
                    # gpsimd is busy with the norm broadcasts here
                    eng = (nc.sync, nc.scalar)[ob % 2]
                else:
                    eng = dma_engines[ob % 3]
                eng.dma_start(
                    out=outT_r[:, ob, th * TH:(th + 1) * TH], in_=ot
                )

            def norm(h, ih, ops):
                hp, hh = divmod(h, 2)
                pb = hh * D
                lrow = small.tile([1, TH], f32, tag="lrow")
                nc.vector.tensor_copy(out=lrow, in_=ops[64:65, :])
                rec = small.tile([1, TH], f32, tag="rec")
                nc.vector.reciprocal_approx_fast(out=rec, in_=lrow)
                bc = small.tile([D, TH], f32, tag="bc")
                nc.gpsimd.partition_broadcast(out_ap=bc, in_ap=rec)
                nc.vector.tensor_mul(
                    out=at[pb:pb + D, hp, ih * TH:(ih + 1) * TH],
                    in0=ops[0:64, :],
                    in1=bc,
                )

            for p_ in range(3):
                pair_s_exp(p_)
                # qk_proj first so the next pair's q/k is never the
                # pair-boundary bottleneck; pv chains fill the rest
                if p_ > 0:
                    qk_proj(p_ + 1, gate=exp_insts[(p_, 0)])
                if p_ > 0:
                    gate = exp_insts[(p_, 0)]
                    pv(2 * p_ - 2, gate=gate)
                    pv(2 * p_ - 1, gate=gate)
            pair3_gate = exp_insts[(2, 7)]
            pv(4, gate=pair3_gate)
            pv(5, gate=pair3_gate)
            # ---- last pair, ih-major with per-ih exps: the ih=0 half of
            # the output projection (th=0 wave, psP) overlaps the ih=1
            # attention half.  Both PV psums live on psB so psP stays free
            # for the wave. ----
            pt3 = ptp.tile([P, 8, 2048], mmdt, tag="pt", name="pt3")
            pts[3] = pt3
            for ih in range(2):
                o6i = psB.tile([P, TH], f32, tag="pv", name=f"o6_{ih}")
                o7i = psB.tile([P, TH], f32, tag="pv", name=f"o7_{ih}")
                sfull = psS.tile([P, 2048], f32, tag="ps", name=f"s3_{ih}")
                p3exp = {}
                for jc in range(8):
                    with tc.high_priority():
                        half = (jc % 2) * 1024
                        sf = sfull[:, half:half + 1024]
                        for pbi, pb in ((0, 0), (1, D)):
                            mm = nc.tensor.matmul(
                                sf[:, pbi * TH:(pbi + 1) * TH],
                                kt[pb:pb + D, 3, jc * P:(jc + 1) * P],
                                qt[pb:pb + D, 3, ih * TH:(ih + 1) * TH],
                                start=True,
                                stop=True,
                                tile_position=(pb, 0),
                            )
                            if jc >= 2:
                                gate_mm(mm, p3exp[jc - 2])
                        p3exp[jc] = nc.scalar.activation(
                            out=pt3[:, jc].rearrange(
                                "p (hh x) -> p hh x", hh=2)
                            [:, :, ih * TH:(ih + 1) * TH],
                            in_=sf, func=Exp, scale=1.0 / 32.0,
                        )
                    nc.tensor.matmul(
                        o6i[0:65],
                        vs[:, jc, 6 * 65:6 * 65 + 65],
                        pt3[:, jc, ih * TH:(ih + 1) * TH],
                        start=(jc == 0),
                        stop=(jc == 7),
                    )
                    nc.tensor.matmul(
                        o7i[0:65],
                        vs[:, jc, 7 * 65:7 * 65 + 65],
                        pt3[:, jc, 1024 + ih * TH: 1024 + (ih + 1) * TH],
                        start=(jc == 0),
                        stop=(jc == 7),
                    )
                norm(6, ih, o6i)
                norm(7, ih, o7i)
                # out-proj wave for this token half (all on psP)
                for ob in range(8):
                    final_group(ob, ih)

    nc.finalize()
    return nc


def _get_nc(mm_dtype=None):
    if mm_dtype is None:
        mm_dtype = MM_DTYPE
    if mm_dtype not in _CACHE:
        _CACHE[mm_dtype] = _build(mm_dtype)
    return _CACHE[mm_dtype]


def make_in_maps(x, Wqkv, bqkv, W1):
    import ml_dtypes
    mmnp = ml_dtypes.bfloat16 if MM_DTYPE == "bfloat16" else np.float32
    x = np.ascontiguousarray(np.asarray(x, dtype=np.float32))
    Wqkv = np.asarray(Wqkv, dtype=np.float32)
    bqkv = np.asarray(bqkv, dtype=np.float32)
    W1 = np.asarray(W1, dtype=np.float32)
    in_maps = []
    for c in range(NCORES):
        b, hg = divmod(c, HG)
        qsl = slice(hg * FQ, (hg + 1) * FQ)
        ksl = slice(DIM + hg * FQ, DIM + (hg + 1) * FQ)
        vsl = slice(2 * DIM + hg * FQ, 2 * DIM + (hg + 1) * FQ)
        q, k, v = Wqkv[qsl], Wqkv[ksl], Wqkv[vsl]
        bq, bk, bvv = bqkv[qsl], bqkv[ksl], bqkv[vsl]
        # pair-major qk interleave: [q_p0, k_p0, q_p1, k_p1, ..., v]
        blocks = []
        bblocks = []
        for p in range(4):
            blocks.append(q[p * P:(p + 1) * P])
            blocks.append(k[p * P:(p + 1) * P])
            bblocks.append(bq[p * P:(p + 1) * P])
            bblocks.append(bk[p * P:(p + 1) * P])
        blocks.append(v)
        bblocks.append(bvv)
        w_loc = np.concatenate(blocks, axis=0)
        b_loc = np.concatenate(bblocks)
        in_maps.append({
            "xT": np.ascontiguousarray(x[b].T.astype(mmnp)),
            "wqkvT": np.ascontiguousarray(w_loc.T.astype(mmnp)),
            "bqkvT": np.ascontiguousarray(b_loc.reshape(FT // P, P).T),
            "bv": np.ascontiguousarray(bqkv[vsl]),
            "w1T": np.ascontiguousarray(W1[:, hg * FQ:(hg + 1) * FQ].T.astype(mmnp)),
        })
    return in_maps


def combine_outputs(results, b1):
    b1 = np.asarray(b1, dtype=np.float32)
    out = np.empty((B, N, DIM), dtype=np.float32)
    for b in range(B):
        acc = (results[HG * b]["outT"].astype(np.float32)
               + results[HG * b + 1]["outT"].astype(np.float32))
        out[b] = acc.T + b1
    return out


def kernel(x, Wqkv, bqkv, W1, b1, trace=False):
    from concourse.bass_utils import run_bass_kernel_spmd

    nc = _get_nc()
    in_maps = make_in_maps(x, Wqkv, bqkv, W1)
    res = run_bass_kernel_spmd(
        nc, in_maps, core_ids=list(range(NCORES)), trace=trace
    )
    out = combine_outputs(res.results, b1)
    if trace:
        kernel.last_result = res
    return out
